# revision 1
# baseline (speedup 1.0000x reference)
"""Trainium2 Bass kernel for 3-layer TransformerConv GNN (heads=1, eval).

Sharding: dst nodes block-sharded over 8 cores (2560 padded nodes each, 20
blocks of 128). Edges routed to dst-owner core, sorted by dst, padded to a
uniform tile count per block so one SPMD program serves all cores. Per layer:
f32r matmuls build a full K|V table + local Q/skip; per 128-edge tile we
indirect-DMA gather q[dst] / k|v[src] rows, DVE mult+reduce logits, ScalarE
exp, and one-hot fp32 matmuls accumulate segment softmax num/den in PSUM.
Between layers the transposed local h shard is AllGathered for the next
K|V table. Biases are zero in setup_inputs and folded out.
"""
import sys

sys.path.insert(0, "/opt/trn_rl_repo")

import numpy as np
from bass_rust import SyncInfo
import concourse.bass as bass
import concourse.mybir as mybir
from concourse.tile import TileContext
from concourse.bass_utils import run_bass_kernel_spmd
from concourse.masks import make_identity

N = 20000
D_IN = 128
DIMS = [(128, 400), (512, 200), (256, 4)]  # (padded f_in, d_out)
DOUT = [400, 200, 4]
NCORES = 8
P = 128
NPAD = 20480
NLOC = NPAD // NCORES      # 2560
NBLK = NLOC // P           # 20
NCHUNK = NPAD // P         # 160

_ctr = [0]


def _split_multi_waits(nc):
    """This walrus build allows only one sync wait per instruction; split
    extras onto single-wait EventSemaphore preludes on the same engine."""
    for f in nc.m.functions:
        for bb in f.blocks:
            out, changed = [], False
            for inst in bb.instructions:
                si = inst.sync_info
                waits = list(si.on_wait) if si is not None else []
                if len(waits) > 1:
                    changed = True
                    for w in waits[:-1]:
                        _ctr[0] += 1
                        out.append(mybir.InstEventSemaphore(
                            name=f"wsplit-{_ctr[0]}", engine=inst.engine,
                            ins=[], outs=[],
                            sync_info=SyncInfo(on_wait=[w], on_update=[])))
                    inst.sync_info = SyncInfo(on_wait=[waits[-1]],
                                              on_update=list(si.on_update))
                out.append(inst)
            if changed:
                bb.instructions = out


def _preprocess(edge_index, T_blk):
    src = np.asarray(edge_index[0], dtype=np.int64)
    dst = np.asarray(edge_index[1], dtype=np.int64)
    NT = NBLK * T_blk
    esrc = np.zeros((NCORES, P, NT), np.int32)
    edst = np.zeros((NCORES, P, NT), np.int32)
    eslot = np.full((NCORES, P, NT), -1.0, np.float32)
    order = np.argsort(dst, kind="stable")
    src_s, dst_s = src[order], dst[order]
    blk = dst_s // P
    core, lblk = blk // NBLK, blk % NBLK
    for c in range(NCORES):
        mc = core == c
        sc, dc, lb = src_s[mc], dst_s[mc], lblk[mc]
        for b in range(NBLK):
            m = lb == b
            s_ids, d_ids = sc[m], dc[m]
            cnt = s_ids.size
            assert cnt <= T_blk * P, (c, b, cnt)
            bt = b * T_blk
            for t in range((cnt + P - 1) // P):
                lo, hi = t * P, min((t + 1) * P, cnt)
                n = hi - lo
                esrc[c, :n, bt + t] = s_ids[lo:hi]
                edst[c, :n, bt + t] = d_ids[lo:hi] - c * NLOC
                eslot[c, :n, bt + t] = (d_ids[lo:hi] % P).astype(np.float32)
    return esrc, edst, eslot


def _build(T_blk):
    NT = NBLK * T_blk
    f32, f32r, i32 = mybir.dt.float32, mybir.dt.float32r, mybir.dt.int32
    AF = mybir.ActivationFunctionType
    OP = mybir.AluOpType
    nc = bass.Bass("TRN2", target_bir_lowering=False, debug=False,
                   num_devices=NCORES)

    xT = nc.declare_dram_parameter("xT", [D_IN, NPAD], f32, isOutput=False)
    xTloc = nc.declare_dram_parameter("xTloc", [D_IN, NLOC], f32,
                                      isOutput=False)
    Wq, Wkv, Ws = [], [], []
    for li, (fp, do) in enumerate(DIMS):
        ch = fp // P
        Wq.append(nc.declare_dram_parameter(f"Wq{li}", [P, ch * do], f32,
                                            isOutput=False))
        Wkv.append(nc.declare_dram_parameter(f"Wkv{li}", [P, ch * 2 * do],
                                             f32, isOutput=False))
        Ws.append(nc.declare_dram_parameter(f"Ws{li}", [P, ch * do], f32,
                                            isOutput=False))
    ESRC = nc.declare_dram_parameter("esrc", [P, NT], i32, isOutput=False)
    EDST = nc.declare_dram_parameter("edst", [P, NT], i32, isOutput=False)
    ESLOT = nc.declare_dram_parameter("eslot", [P, NT], f32, isOutput=False)
    OUT = nc.declare_dram_parameter("out", [NLOC, DOUT[2]], f32,
                                    isOutput=True)

    KV = [nc.dram_tensor(f"KV{li}", [NPAD, 2 * DOUT[li]], f32)
          for li in range(3)]
    QL = [nc.dram_tensor(f"QL{li}", [NLOC, DOUT[li]], f32) for li in range(3)]
    SL = [nc.dram_tensor(f"SL{li}", [NLOC, DOUT[li]], f32) for li in range(3)]
    hTl = [nc.dram_tensor(f"hTl{li}", [DIMS[li + 1][0], NLOC], f32)
           for li in range(2)]
    hTg = [nc.dram_tensor(f"hTg{li}", [NCORES * DIMS[li + 1][0], NLOC], f32,
                          addr_space="Shared") for li in range(2)]

    with TileContext(nc) as tc:
        with (
            tc.tile_pool(name="const", bufs=1) as cpool,
            tc.tile_pool(name="w", bufs=1) as wpool,
            tc.tile_pool(name="lhs", bufs=3) as lhspool,
            tc.tile_pool(name="tab", bufs=3) as tabpool,
            tc.tile_pool(name="tps", bufs=1, space="PSUM") as tps,
            tc.tile_pool(name="edge", bufs=4) as ep,
            tc.tile_pool(name="seg", bufs=2, space="PSUM") as segps,
            tc.tile_pool(name="blk", bufs=2) as bp,
            tc.tile_pool(name="tr", bufs=2, space="PSUM") as trps,
        ):
            ident = cpool.tile([P, P], f32)
            make_identity(nc, ident[:])
            iot = cpool.tile([P, P], i32)
            nc.gpsimd.iota(iot[:], pattern=[[1, P]], base=0,
                           channel_multiplier=0)
            iotf = cpool.tile([P, P], f32)
            nc.vector.tensor_copy(iotf[:], iot[:])
            ones = cpool.tile([P, 1], f32)
            nc.gpsimd.memset(ones[:], 1.0)
            esrc_sb = cpool.tile([P, NT], i32)
            nc.sync.dma_start(esrc_sb[:], ESRC[:])
            edst_sb = cpool.tile([P, NT], i32)
            nc.sync.dma_start(edst_sb[:], EDST[:])
            eslot_sb = cpool.tile([P, NT], f32)
            nc.sync.dma_start(eslot_sb[:], ESLOT[:])
            wq_sb, wkv_sb, ws_sb = [], [], []
            for li, (fp, do) in enumerate(DIMS):
                ch = fp // P
                t1 = wpool.tile([P, ch * do], f32r, tag=f"wq{li}")
                nc.gpsimd.dma_start(t1[:], Wq[li][:])
                t2 = wpool.tile([P, ch * 2 * do], f32r, tag=f"wkv{li}")
                nc.gpsimd.dma_start(t2[:], Wkv[li][:])
                t3 = wpool.tile([P, ch * do], f32r, tag=f"ws{li}")
                nc.gpsimd.dma_start(t3[:], Ws[li][:])
                wq_sb.append(t1)
                wkv_sb.append(t2)
                ws_sb.append(t3)

            for li, (fp, do) in enumerate(DIMS):
                ch = fp // P
                scale = float(1.0 / np.sqrt(do))

                def load_lhs(cglob):
                    """f32r tile [P, ch, P]: [f_in_chunk, fc, node]."""
                    t = lhspool.tile([P, ch, P], f32r, tag="lhs")
                    if li == 0:
                        nc.gpsimd.dma_start(
                            t[:, 0, :], xT[:, cglob * P:(cglob + 1) * P])
                    else:
                        r, cl = divmod(cglob, NBLK)
                        src = hTg[li - 1][r * fp:(r + 1) * fp,
                                          cl * P:(cl + 1) * P]
                        nc.gpsimd.dma_start(
                            t[:], src.rearrange("(c p) n -> p c n", p=P))
                    return t

                def load_lhs_loc(cl):
                    t = lhspool.tile([P, ch, P], f32r, tag="lhs")
                    if li == 0:
                        nc.gpsimd.dma_start(
                            t[:, 0, :], xTloc[:, cl * P:(cl + 1) * P])
                    else:
                        src = hTl[li - 1][:, cl * P:(cl + 1) * P]
                        nc.gpsimd.dma_start(
                            t[:], src.rearrange("(c p) n -> p c n", p=P))
                    return t

                # ---- full K|V table ----
                for cg in range(NCHUNK):
                    lt = load_lhs(cg)
                    pk = tps.tile([P, do], f32, tag="pk", space="PSUM")
                    pv = tps.tile([P, do], f32, tag="pv", space="PSUM")
                    for fc in range(ch):
                        w = wkv_sb[li]
                        nc.tensor.matmul(
                            pk[:], lhsT=lt[:, fc, :],
                            rhs=w[:, fc * 2 * do:fc * 2 * do + do],
                            start=(fc == 0), stop=(fc == ch - 1))
                        nc.tensor.matmul(
                            pv[:], lhsT=lt[:, fc, :],
                            rhs=w[:, fc * 2 * do + do:(fc + 1) * 2 * do],
                            start=(fc == 0), stop=(fc == ch - 1))
                    kvt = tabpool.tile([P, 2 * do], f32, tag="kvt")
                    nc.vector.tensor_copy(kvt[:, :do], pk[:])
                    nc.vector.tensor_copy(kvt[:, do:], pv[:])
                    nc.sync.dma_start(KV[li][cg * P:(cg + 1) * P, :], kvt[:])

                # ---- local Q and skip ----
                for cl in range(NBLK):
                    lt = load_lhs_loc(cl)
                    pq = tps.tile([P, do], f32, tag="pk", space="PSUM")
                    pv = tps.tile([P, do], f32, tag="pv", space="PSUM")
                    for fc in range(ch):
                        nc.tensor.matmul(
                            pq[:], lhsT=lt[:, fc, :],
                            rhs=wq_sb[li][:, fc * do:(fc + 1) * do],
                            start=(fc == 0), stop=(fc == ch - 1))
                        nc.tensor.matmul(
                            pv[:], lhsT=lt[:, fc, :],
                            rhs=ws_sb[li][:, fc * do:(fc + 1) * do],
                            start=(fc == 0), stop=(fc == ch - 1))
                    qt = tabpool.tile([P, do], f32, tag="qt")
                    nc.vector.tensor_copy(qt[:], pq[:])
                    nc.sync.dma_start(QL[li][cl * P:(cl + 1) * P, :], qt[:])
                    st = tabpool.tile([P, do], f32, tag="st")
                    nc.vector.tensor_copy(st[:], pv[:])
                    nc.sync.dma_start(SL[li][cl * P:(cl + 1) * P, :], st[:])

                # ---- edge phase ----
                for b in range(NBLK):
                    seg = segps.tile([P, do], f32, tag="seg",
                                     space="PSUM")
                    segd = segps.tile([P, 1], f32, tag="segd",
                                      space="PSUM")
                    for t in range(T_blk):
                        gt = b * T_blk + t
                        kvg = ep.tile([P, 2 * do], f32, tag="kvg")
                        nc.gpsimd.indirect_dma_start(
                            out=kvg[:], out_offset=None, in_=KV[li][:],
                            in_offset=bass.IndirectOffsetOnAxis(
                                ap=esrc_sb[:, gt:gt + 1], axis=0))
                        qg = ep.tile([P, do], f32, tag="qg")
                        nc.gpsimd.indirect_dma_start(
                            out=qg[:], out_offset=None, in_=QL[li][:],
                            in_offset=bass.IndirectOffsetOnAxis(
                                ap=edst_sb[:, gt:gt + 1], axis=0))
                        prod = ep.tile([P, do], f32, tag="prod")
                        nc.vector.tensor_tensor(out=prod[:], in0=qg[:],
                                                in1=kvg[:, :do], op=OP.mult)
                        lcol = ep.tile([P, 1], f32, tag="lcol")
                        nc.vector.tensor_reduce(out=lcol[:], in_=prod[:],
                                                axis=mybir.AxisListType.X,
                                                op=OP.add)
                        ecol = ep.tile([P, 1], f32, tag="ecol")
                        nc.scalar.activation(ecol[:], lcol[:], AF.Exp,
                                             scale=scale)
                        oh = ep.tile([P, P], f32, tag="oh")
                        nc.vector.tensor_scalar(
                            out=oh[:], in0=iotf[:],
                            scalar1=eslot_sb[:, gt:gt + 1], scalar2=None,
                            op0=OP.is_equal)
                        M = ep.tile([P, P], f32, tag="M")
                        nc.vector.tensor_scalar(
                            out=M[:], in0=oh[:], scalar1=ecol[:, :1],
                            scalar2=None, op0=OP.mult)
                        nc.tensor.matmul(seg[:, :do], lhsT=M[:],
                                         rhs=kvg[:, do:2 * do],
                                         start=(t == 0), stop=(t == T_blk - 1))
                        nc.tensor.matmul(segd[:, :1], lhsT=M[:],
                                         rhs=ones[:], start=(t == 0),
                                         stop=(t == T_blk - 1))

                    dcol = bp.tile([P, 1], f32, tag="dcol")
                    nc.vector.tensor_copy(dcol[:], segd[:, :1])
                    rden = bp.tile([P, 1], f32, tag="rden")
                    nc.vector.reciprocal(rden[:], dcol[:])
                    aggs = bp.tile([P, do], f32, tag="aggs")
                    nc.vector.tensor_scalar(
                        out=aggs[:], in0=seg[:, :do], scalar1=rden[:, :1],
                        scalar2=None, op0=OP.mult)
                    skb = bp.tile([P, do], f32, tag="skb")
                    nc.sync.dma_start(skb[:],
                                      SL[li][b * P:(b + 1) * P, :])
                    hsum = bp.tile([P, do], f32, tag="hsum")
                    nc.vector.tensor_tensor(out=hsum[:], in0=aggs[:],
                                            in1=skb[:], op=OP.add)
                    if li == 2:
                        hout = bp.tile([P, do], f32, tag="hout")
                        nc.scalar.activation(hout[:], hsum[:], AF.Relu)
                        nc.sync.dma_start(OUT[b * P:(b + 1) * P, :], hout[:])
                    else:
                        fpn = DIMS[li + 1][0]
                        hpad = bp.tile([P, fpn], f32, tag="hpad")
                        nc.gpsimd.memset(hpad[:], 0.0)
                        nc.scalar.activation(hpad[:, :do], hsum[:], AF.Relu)
                        for fc2 in range(fpn // P):
                            tp = trps.tile([P, P], f32, tag="tp",
                                           space="PSUM")
                            nc.tensor.transpose(
                                tp[:], hpad[:, fc2 * P:(fc2 + 1) * P],
                                ident[:])
                            hts = bp.tile([P, P], f32, tag="hts")
                            nc.vector.tensor_copy(hts[:], tp[:])
                            nc.sync.dma_start(
                                hTl[li][fc2 * P:(fc2 + 1) * P,
                                        b * P:(b + 1) * P], hts[:])

                if li < 2:
                    nc.gpsimd.collective_compute(
                        "AllGather", mybir.AluOpType.bypass,
                        replica_groups=[list(range(NCORES))],
                        ins=[hTl[li][:]], outs=[hTg[li][:]])

    _split_multi_waits(nc)
    return nc


_CACHE = {}


def kernel(**inputs):
    x = np.asarray(inputs["x"], dtype=np.float32)
    edge_index = np.asarray(inputs["edge_index"])

    # tile count per dst block (uniform across cores for one SPMD program)
    dst = edge_index[1].astype(np.int64)
    cnt = np.bincount(dst // P, minlength=NCHUNK)
    T_blk = int(np.ceil(cnt.max() / P))
    esrc, edst, eslot = _preprocess(edge_index, T_blk)

    xT = np.zeros((D_IN, NPAD), np.float32)
    xT[:, :N] = x.T

    wq_in, wkv_in, ws_in = [], [], []
    for li, (fp, do) in enumerate(DIMS):
        ch = fp // P
        l = li + 1
        din = [128, 400, 200][li]

        def pack(w):
            wp = np.zeros((fp, do), np.float32)
            wp[:din] = np.asarray(w, dtype=np.float32)
            return np.ascontiguousarray(
                wp.reshape(ch, P, do).transpose(1, 0, 2).reshape(P, ch * do))

        wq_in.append(pack(inputs[f"Wq{l}"]))
        ws_in.append(pack(inputs[f"Ws{l}"]))
        wk = np.zeros((fp, do), np.float32)
        wk[:din] = np.asarray(inputs[f"Wk{l}"], dtype=np.float32)
        wv = np.zeros((fp, do), np.float32)
        wv[:din] = np.asarray(inputs[f"Wv{l}"], dtype=np.float32)
        wkv = np.concatenate([wk.reshape(ch, P, do),
                              wv.reshape(ch, P, do)], axis=2)  # [ch,P,2do]
        wkv_in.append(np.ascontiguousarray(
            wkv.transpose(1, 0, 2).reshape(P, ch * 2 * do)))

    if T_blk not in _CACHE:
        _CACHE[T_blk] = _build(T_blk)
    nc = _CACHE[T_blk]

    in_maps = []
    for c in range(NCORES):
        m = dict(xT=xT, xTloc=np.ascontiguousarray(
            xT[:, c * NLOC:(c + 1) * NLOC]),
            esrc=esrc[c], edst=edst[c], eslot=eslot[c])
        for li in range(3):
            m[f"Wq{li}"] = wq_in[li]
            m[f"Wkv{li}"] = wkv_in[li]
            m[f"Ws{li}"] = ws_in[li]
        in_maps.append(m)

    res = run_bass_kernel_spmd(nc, in_maps, list(range(NCORES)))
    out = np.concatenate([res.results[c]["out"] for c in range(NCORES)],
                         axis=0)
    return np.ascontiguousarray(out[:N]).astype(np.float32)



# revision 30
# speedup vs baseline: 5202.1875x; 5202.1875x over previous
"""Trainium2 Bass kernel for 3-layer TransformerConv GNN (heads=1, eval).

Sharding: dst nodes block-sharded over 8 cores (2560 padded nodes each, 20
blocks of 128). Edges routed to dst-owner core, sorted by dst, padded to a
uniform tile count per block so one SPMD program serves all cores. Per layer:
f32r matmuls build a full K|V table + local Q/skip; per 128-edge tile we
indirect-DMA gather q[dst] / k|v[src] rows, DVE mult+reduce logits, ScalarE
exp, and one-hot fp32 matmuls accumulate segment softmax num/den in PSUM.
Between layers the transposed local h shard is AllGathered for the next
K|V table. Biases are zero in setup_inputs and folded out.
"""
import sys

sys.path.insert(0, "/opt/trn_rl_repo")

import numpy as np
from bass_rust import SyncInfo
import concourse.bass as bass
import concourse.mybir as mybir
from concourse.tile import TileContext
from concourse.bass_utils import run_bass_kernel_spmd
from concourse.masks import make_identity

N = 20000
D_IN = 128
DIMS = [(128, 400), (512, 200), (256, 4)]  # (padded f_in, d_out)
DOUT = [400, 200, 4]
NCORES = 8
P = 128
NPAD = 20480
NLOC = NPAD // NCORES      # 2560
NBLK = NLOC // P           # 20
NCHUNK = NPAD // P         # 160

_ctr = [0]


def _split_multi_waits(nc):
    """This walrus build allows only one sync wait per instruction; split
    extras onto single-wait EventSemaphore preludes on the same engine."""
    for f in nc.m.functions:
        for bb in f.blocks:
            out, changed = [], False
            for inst in bb.instructions:
                si = inst.sync_info
                waits = list(si.on_wait) if si is not None else []
                if len(waits) > 1:
                    changed = True
                    for w in waits[:-1]:
                        _ctr[0] += 1
                        out.append(mybir.InstEventSemaphore(
                            name=f"wsplit-{_ctr[0]}", engine=inst.engine,
                            ins=[], outs=[],
                            sync_info=SyncInfo(on_wait=[w], on_update=[])))
                    inst.sync_info = SyncInfo(on_wait=[waits[-1]],
                                              on_update=list(si.on_update))
                out.append(inst)
            if changed:
                bb.instructions = out


def _preprocess(edge_index, T_blk):
    src = np.asarray(edge_index[0], dtype=np.int64)
    dst = np.asarray(edge_index[1], dtype=np.int64)
    NT = NBLK * T_blk
    esrc = np.zeros((NCORES, P, NT), np.int32)
    edst = np.zeros((NCORES, P, NT), np.int32)
    eslot = np.full((NCORES, P, NT), -1.0, np.float32)
    order = np.argsort(dst, kind="stable")
    src_s, dst_s = src[order], dst[order]
    blk = dst_s // P
    core, lblk = blk // NBLK, blk % NBLK
    for c in range(NCORES):
        mc = core == c
        sc, dc, lb = src_s[mc], dst_s[mc], lblk[mc]
        for b in range(NBLK):
            m = lb == b
            s_ids, d_ids = sc[m], dc[m]
            cnt = s_ids.size
            assert cnt <= T_blk * P, (c, b, cnt)
            bt = b * T_blk
            for t in range((cnt + P - 1) // P):
                lo, hi = t * P, min((t + 1) * P, cnt)
                n = hi - lo
                esrc[c, :n, bt + t] = s_ids[lo:hi]
                edst[c, :n, bt + t] = d_ids[lo:hi] - c * NLOC
                eslot[c, :n, bt + t] = (d_ids[lo:hi] % P).astype(np.float32)
    return esrc, edst, eslot


def _build(T_blk):
    NT = NBLK * T_blk
    f32, f32r, i32 = mybir.dt.float32, mybir.dt.float32r, mybir.dt.int32
    AF = mybir.ActivationFunctionType
    OP = mybir.AluOpType
    nc = bass.Bass("TRN2", target_bir_lowering=False, debug=False,
                   num_devices=NCORES)

    xT = nc.declare_dram_parameter("xT", [D_IN, NPAD], f32, isOutput=False)
    xTloc = nc.declare_dram_parameter("xTloc", [D_IN, NLOC], f32,
                                      isOutput=False)
    Wq, Wkv, Ws = [], [], []
    for li, (fp, do) in enumerate(DIMS):
        ch = fp // P
        Wq.append(nc.declare_dram_parameter(f"Wq{li}", [P, ch * do], f32,
                                            isOutput=False))
        Wkv.append(nc.declare_dram_parameter(f"Wkv{li}", [P, ch * 2 * do],
                                             f32, isOutput=False))
        Ws.append(nc.declare_dram_parameter(f"Ws{li}", [P, ch * do], f32,
                                            isOutput=False))
    ESRC = nc.declare_dram_parameter("esrc", [P, NT], i32, isOutput=False)
    EDST = nc.declare_dram_parameter("edst", [P, NT], i32, isOutput=False)
    ESLOT = nc.declare_dram_parameter("eslot", [P, NT], f32, isOutput=False)
    OUT = nc.declare_dram_parameter("out", [NLOC, DOUT[2]], f32,
                                    isOutput=True)

    KV = [nc.dram_tensor(f"KV{li}", [NPAD, 2 * DOUT[li]], f32)
          for li in range(3)]
    QL = [nc.dram_tensor(f"QL{li}", [NLOC, DOUT[li]], f32) for li in range(3)]
    SL = [nc.dram_tensor(f"SL{li}", [NLOC, DOUT[li]], f32) for li in range(3)]
    hTl = [nc.dram_tensor(f"hTl{li}", [DIMS[li + 1][0], NLOC], f32)
           for li in range(2)]
    hTg = [nc.dram_tensor(f"hTg{li}", [NCORES * DIMS[li + 1][0], NLOC], f32,
                          addr_space="Shared") for li in range(2)]

    with TileContext(nc) as tc:
        with (
            tc.tile_pool(name="const", bufs=1) as cpool,
            tc.tile_pool(name="w", bufs=1) as wpool,
            tc.tile_pool(name="lhs", bufs=3) as lhspool,
            tc.tile_pool(name="tab", bufs=3) as tabpool,
            tc.tile_pool(name="tps", bufs=1, space="PSUM") as tps,
            tc.tile_pool(name="edge", bufs=4) as ep,
            tc.tile_pool(name="seg", bufs=2, space="PSUM") as segps,
            tc.tile_pool(name="blk", bufs=2) as bp,
            tc.tile_pool(name="tr", bufs=2, space="PSUM") as trps,
        ):
            ident = cpool.tile([P, P], f32)
            make_identity(nc, ident[:])
            iot = cpool.tile([P, P], i32)
            nc.gpsimd.iota(iot[:], pattern=[[1, P]], base=0,
                           channel_multiplier=0)
            iotf = cpool.tile([P, P], f32)
            nc.vector.tensor_copy(iotf[:], iot[:])
            ones = cpool.tile([P, 1], f32)
            nc.gpsimd.memset(ones[:], 1.0)
            esrc_sb = cpool.tile([P, NT], i32)
            nc.sync.dma_start(esrc_sb[:], ESRC[:])
            edst_sb = cpool.tile([P, NT], i32)
            nc.sync.dma_start(edst_sb[:], EDST[:])
            eslot_sb = cpool.tile([P, NT], f32)
            nc.sync.dma_start(eslot_sb[:], ESLOT[:])
            wq_sb, wkv_sb, ws_sb = [], [], []
            for li, (fp, do) in enumerate(DIMS):
                ch = fp // P
                t1 = wpool.tile([P, ch * do], f32r, tag=f"wq{li}")
                nc.gpsimd.dma_start(t1[:], Wq[li][:])
                t2 = wpool.tile([P, ch * 2 * do], f32r, tag=f"wkv{li}")
                nc.gpsimd.dma_start(t2[:], Wkv[li][:])
                t3 = wpool.tile([P, ch * do], f32r, tag=f"ws{li}")
                nc.gpsimd.dma_start(t3[:], Ws[li][:])
                wq_sb.append(t1)
                wkv_sb.append(t2)
                ws_sb.append(t3)

            for li, (fp, do) in enumerate(DIMS):
                ch = fp // P
                scale = float(1.0 / np.sqrt(do))

                def load_lhs(cglob):
                    """f32r tile [P, ch, P]: [f_in_chunk, fc, node]."""
                    t = lhspool.tile([P, ch, P], f32r, tag="lhs")
                    if li == 0:
                        nc.gpsimd.dma_start(
                            t[:, 0, :], xT[:, cglob * P:(cglob + 1) * P])
                    else:
                        r, cl = divmod(cglob, NBLK)
                        src = hTg[li - 1][r * fp:(r + 1) * fp,
                                          cl * P:(cl + 1) * P]
                        nc.gpsimd.dma_start(
                            t[:], src.rearrange("(c p) n -> p c n", p=P))
                    return t

                def load_lhs_loc(cl):
                    t = lhspool.tile([P, ch, P], f32r, tag="lhs")
                    if li == 0:
                        nc.gpsimd.dma_start(
                            t[:, 0, :], xTloc[:, cl * P:(cl + 1) * P])
                    else:
                        src = hTl[li - 1][:, cl * P:(cl + 1) * P]
                        nc.gpsimd.dma_start(
                            t[:], src.rearrange("(c p) n -> p c n", p=P))
                    return t

                # ---- full K|V table ----
                for cg in range(NCHUNK):
                    lt = load_lhs(cg)
                    pk = tps.tile([P, do], f32, tag="pk", space="PSUM")
                    pv = tps.tile([P, do], f32, tag="pv", space="PSUM")
                    for fc in range(ch):
                        w = wkv_sb[li]
                        nc.tensor.matmul(
                            pk[:], lhsT=lt[:, fc, :],
                            rhs=w[:, fc * 2 * do:fc * 2 * do + do],
                            start=(fc == 0), stop=(fc == ch - 1))
                        nc.tensor.matmul(
                            pv[:], lhsT=lt[:, fc, :],
                            rhs=w[:, fc * 2 * do + do:(fc + 1) * 2 * do],
                            start=(fc == 0), stop=(fc == ch - 1))
                    kvt = tabpool.tile([P, 2 * do], f32, tag="kvt")
                    nc.vector.tensor_copy(kvt[:, :do], pk[:])
                    nc.vector.tensor_copy(kvt[:, do:], pv[:])
                    nc.sync.dma_start(KV[li][cg * P:(cg + 1) * P, :], kvt[:])

                # ---- local Q and skip ----
                for cl in range(NBLK):
                    lt = load_lhs_loc(cl)
                    pq = tps.tile([P, do], f32, tag="pk", space="PSUM")
                    pv = tps.tile([P, do], f32, tag="pv", space="PSUM")
                    for fc in range(ch):
                        nc.tensor.matmul(
                            pq[:], lhsT=lt[:, fc, :],
                            rhs=wq_sb[li][:, fc * do:(fc + 1) * do],
                            start=(fc == 0), stop=(fc == ch - 1))
                        nc.tensor.matmul(
                            pv[:], lhsT=lt[:, fc, :],
                            rhs=ws_sb[li][:, fc * do:(fc + 1) * do],
                            start=(fc == 0), stop=(fc == ch - 1))
                    qt = tabpool.tile([P, do], f32, tag="qt")
                    nc.vector.tensor_copy(qt[:], pq[:])
                    nc.sync.dma_start(QL[li][cl * P:(cl + 1) * P, :], qt[:])
                    st = tabpool.tile([P, do], f32, tag="st")
                    nc.vector.tensor_copy(st[:], pv[:])
                    nc.sync.dma_start(SL[li][cl * P:(cl + 1) * P, :], st[:])

                # ---- edge phase ----
                for b in range(NBLK):
                    seg = segps.tile([P, do], f32, tag="seg",
                                     space="PSUM")
                    segd = segps.tile([P, 1], f32, tag="segd",
                                      space="PSUM")
                    for t in range(T_blk):
                        gt = b * T_blk + t
                        kvg = ep.tile([P, 2 * do], f32, tag="kvg")
                        nc.gpsimd.indirect_dma_start(
                            out=kvg[:], out_offset=None, in_=KV[li][:],
                            in_offset=bass.IndirectOffsetOnAxis(
                                ap=esrc_sb[:, gt:gt + 1], axis=0))
                        qg = ep.tile([P, do], f32, tag="qg")
                        nc.gpsimd.indirect_dma_start(
                            out=qg[:], out_offset=None, in_=QL[li][:],
                            in_offset=bass.IndirectOffsetOnAxis(
                                ap=edst_sb[:, gt:gt + 1], axis=0))
                        prod = ep.tile([P, do], f32, tag="prod")
                        nc.vector.tensor_tensor(out=prod[:], in0=qg[:],
                                                in1=kvg[:, :do], op=OP.mult)
                        lcol = ep.tile([P, 1], f32, tag="lcol")
                        nc.vector.tensor_reduce(out=lcol[:], in_=prod[:],
                                                axis=mybir.AxisListType.X,
                                                op=OP.add)
                        ecol = ep.tile([P, 1], f32, tag="ecol")
                        nc.scalar.activation(ecol[:], lcol[:], AF.Exp,
                                             scale=scale)
                        oh = ep.tile([P, P], f32, tag="oh")
                        nc.vector.tensor_scalar(
                            out=oh[:], in0=iotf[:],
                            scalar1=eslot_sb[:, gt:gt + 1], scalar2=None,
                            op0=OP.is_equal)
                        M = ep.tile([P, P], f32, tag="M")
                        nc.vector.tensor_scalar(
                            out=M[:], in0=oh[:], scalar1=ecol[:, :1],
                            scalar2=None, op0=OP.mult)
                        nc.tensor.matmul(seg[:, :do], lhsT=M[:],
                                         rhs=kvg[:, do:2 * do],
                                         start=(t == 0), stop=(t == T_blk - 1))
                        nc.tensor.matmul(segd[:, :1], lhsT=M[:],
                                         rhs=ones[:], start=(t == 0),
                                         stop=(t == T_blk - 1))

                    dcol = bp.tile([P, 1], f32, tag="dcol")
                    nc.vector.tensor_copy(dcol[:], segd[:, :1])
                    rden = bp.tile([P, 1], f32, tag="rden")
                    nc.vector.reciprocal(rden[:], dcol[:])
                    aggs = bp.tile([P, do], f32, tag="aggs")
                    nc.vector.tensor_scalar(
                        out=aggs[:], in0=seg[:, :do], scalar1=rden[:, :1],
                        scalar2=None, op0=OP.mult)
                    skb = bp.tile([P, do], f32, tag="skb")
                    nc.sync.dma_start(skb[:],
                                      SL[li][b * P:(b + 1) * P, :])
                    hsum = bp.tile([P, do], f32, tag="hsum")
                    nc.vector.tensor_tensor(out=hsum[:], in0=aggs[:],
                                            in1=skb[:], op=OP.add)
                    if li == 2:
                        hout = bp.tile([P, do], f32, tag="hout")
                        nc.scalar.activation(hout[:], hsum[:], AF.Relu)
                        nc.sync.dma_start(OUT[b * P:(b + 1) * P, :], hout[:])
                    else:
                        fpn = DIMS[li + 1][0]
                        hpad = bp.tile([P, fpn], f32, tag="hpad")
                        nc.gpsimd.memset(hpad[:], 0.0)
                        nc.scalar.activation(hpad[:, :do], hsum[:], AF.Relu)
                        for fc2 in range(fpn // P):
                            tp = trps.tile([P, P], f32, tag="tp",
                                           space="PSUM")
                            nc.tensor.transpose(
                                tp[:], hpad[:, fc2 * P:(fc2 + 1) * P],
                                ident[:])
                            hts = bp.tile([P, P], f32, tag="hts")
                            nc.vector.tensor_copy(hts[:], tp[:])
                            nc.sync.dma_start(
                                hTl[li][fc2 * P:(fc2 + 1) * P,
                                        b * P:(b + 1) * P], hts[:])

                if li < 2:
                    nc.gpsimd.collective_compute(
                        "AllGather", mybir.AluOpType.bypass,
                        replica_groups=[list(range(NCORES))],
                        ins=[hTl[li][:]], outs=[hTg[li][:]])

    _split_multi_waits(nc)
    return nc


_CACHE = {}


def _prepare(inputs):
    x = np.asarray(inputs["x"], dtype=np.float32)
    edge_index = np.asarray(inputs["edge_index"])

    # tile count per dst block (uniform across cores for one SPMD program)
    dst = edge_index[1].astype(np.int64)
    cnt = np.bincount(dst // P, minlength=NCHUNK)
    T_blk = int(np.ceil(cnt.max() / P))
    esrc, edst, eslot = _preprocess(edge_index, T_blk)

    xT = np.zeros((D_IN, NPAD), np.float32)
    xT[:, :N] = x.T

    wq_in, wkv_in, ws_in = [], [], []
    for li, (fp, do) in enumerate(DIMS):
        ch = fp // P
        l = li + 1
        din = [128, 400, 200][li]

        def pack(w):
            wp = np.zeros((fp, do), np.float32)
            wp[:din] = np.asarray(w, dtype=np.float32)
            return np.ascontiguousarray(
                wp.reshape(ch, P, do).transpose(1, 0, 2).reshape(P, ch * do))

        wq_in.append(pack(inputs[f"Wq{l}"]))
        ws_in.append(pack(inputs[f"Ws{l}"]))
        wk = np.zeros((fp, do), np.float32)
        wk[:din] = np.asarray(inputs[f"Wk{l}"], dtype=np.float32)
        wv = np.zeros((fp, do), np.float32)
        wv[:din] = np.asarray(inputs[f"Wv{l}"], dtype=np.float32)
        wkv = np.concatenate([wk.reshape(ch, P, do),
                              wv.reshape(ch, P, do)], axis=2)  # [ch,P,2do]
        wkv_in.append(np.ascontiguousarray(
            wkv.transpose(1, 0, 2).reshape(P, ch * 2 * do)))

    if T_blk not in _CACHE:
        _CACHE[T_blk] = _build(T_blk)
    nc = _CACHE[T_blk]

    in_maps = []
    for c in range(NCORES):
        m = dict(xT=xT, xTloc=np.ascontiguousarray(
            xT[:, c * NLOC:(c + 1) * NLOC]),
            esrc=esrc[c], edst=edst[c], eslot=eslot[c])
        for li in range(3):
            m[f"Wq{li}"] = wq_in[li]
            m[f"Wkv{li}"] = wkv_in[li]
            m[f"Ws{li}"] = ws_in[li]
        in_maps.append(m)
    return nc, in_maps


def kernel(**inputs):
    nc, in_maps = _prepare(inputs)
    res = run_bass_kernel_spmd(nc, in_maps, list(range(NCORES)))
    global _LAST_RES
    _LAST_RES = res
    out = np.concatenate([res.results[c]["out"] for c in range(NCORES)],
                         axis=0)
    return np.ascontiguousarray(out[:N]).astype(np.float32)


_LAST_RES = None

